# revision 8
# baseline (speedup 1.0000x reference)
"""Trainium2 Bass kernel for nn_LutLinear (BCQ/LUT-quantized linear layer).

Math (K=4096, N=4096, WBIT=3, GROUP=128, APOT=3):
  bits[k, b, n]  = bit (k%32) of binaryWeight[k//32, b, n]
  B              = 2*bits - 1                        (in {-1, +1})
  scale[n, b, g] = sum_a 2^alpha[n, b, g, a]
  out[n] = sum_{g,b} scale[n,b,g] * (sum_{k in group g} x[k] * B[k,b,n]) + bias[n]

Strategy (tensor-parallel over N, 8 cores, N'=512 each):
  * DVE bit-unpack: (words << t) & 0x4040 on int16 lanes yields fp8e4 planes
    (byte 0x40 = 2.0) -- 2 bit-planes per int16 element, 8 shift values,
    split per b-slice for DMA/PE pipelining (24 ops).
  * PE: 48 fp8 DoubleRow matmuls.  Each pairs k-tiles (j, j+8) and carries a
    64-wide stationary bank: cols 0-31 = Q(x) (fp8 of x), cols 32-63 = R
    (fp8 of the quantization residual), so full ~bf16 x-precision at fp8
    DoubleRow speed.  psum[64b+32h+g, n'] = 2*sum_k Qh_k*bit.
  * Tail: prod = psum * scale (bf16, DVE), then ones^T @ prod on PE reduces
    over the 192 (b,h,g) rows; bias2 = bias - sum_q scale*S_g rides as a
    65th rhs row.  Output [1, 512] f32 DMA'd straight from PSUM.
"""

import os
import sys

for _p in ("/opt/trn_rl_repo", "/opt/pypackages"):
    if os.path.isdir(_p) and _p not in sys.path:
        sys.path.insert(0, _p)

from contextlib import ExitStack

import ml_dtypes
import numpy as np

import concourse.bass as bass
import concourse.tile as tile
from concourse import bacc, mybir
from concourse._compat import with_exitstack
from concourse.bass_utils import run_bass_kernel_spmd

K = 4096
N = 4096
GROUP = 128
WBIT = 3
NUM_APOT = 3
G = K // GROUP          # 32 groups
NCORES = 8
NS = N // NCORES        # 512 output features per core
WORDS = K // 32         # 128 packed words per (b, n)
F8 = ml_dtypes.float8_e4m3
BF16 = ml_dtypes.bfloat16

WARM_N = 6              # PE p-state warm-up matmuls

_CACHE = {}


@with_exitstack
def _build_kernel_body(ctx: ExitStack, tc):
    nc = tc.nc
    f32 = mybir.dt.float32
    i32 = mybir.dt.int32
    i16 = mybir.dt.int16
    bf16 = mybir.dt.bfloat16
    f8 = mybir.dt.float8e4
    DR = mybir.MatmulPerfMode.DoubleRow

    bw = nc.dram_tensor("bw", [WORDS, WBIT * NS], i32, kind="ExternalInput")
    xall = nc.dram_tensor("xall", [WORDS, 2048], f8, kind="ExternalInput")
    consts = nc.dram_tensor("consts", [64, 3 * NS], bf16, kind="ExternalInput")
    bias2 = nc.dram_tensor("bias2", [1, NS], bf16, kind="ExternalInput")
    out = nc.dram_tensor("out", [1, NS], f32, kind="ExternalOutput")

    sb = ctx.enter_context(tc.tile_pool(name="sb", bufs=1))
    psum_a = ctx.enter_context(tc.tile_pool(name="psum_a", bufs=1, space="PSUM"))
    psum_b = ctx.enter_context(tc.tile_pool(name="psum_b", bufs=1, space="PSUM"))
    psum_c = ctx.enter_context(tc.tile_pool(name="psum_c", bufs=1, space="PSUM"))
    psum_o = ctx.enter_context(tc.tile_pool(name="psum_o", bufs=1, space="PSUM"))

    # --- input DMAs (three parallel HWDGE queues; bw slices gate the pipe) ---
    bw3 = bw[:, :].rearrange("p (b n) -> p b n", b=WBIT)
    wsb = sb.tile([WORDS, WBIT * NS], i32)
    wsb3 = wsb[:].rearrange("p (b n) -> p b n", b=WBIT)
    nc.sync.dma_start(wsb3[:, 0, :], bw3[:, 0, :])
    xsb = sb.tile([WORDS, 2048], f8)
    nc.scalar.dma_start(xsb[:], xall[:, :])
    nc.scalar.dma_start(wsb3[:, 1, :], bw3[:, 1, :])
    nc.sync.dma_start(wsb3[:, 2, :], bw3[:, 2, :])
    csb = sb.tile([64, 3 * NS], bf16)
    nc.scalar.dma_start(csb[:], consts[:, :])
    pr2 = sb.tile([65, NS], bf16)
    nc.sync.dma_start(pr2[64:65, :], bias2[0:1, :])

    # --- PE p-state warm-up during the DMA wait -------------------------------
    warm = sb.tile([128, 512], bf16)
    nc.gpsimd.memset(warm[:], 0.0)
    ones_t = sb.tile([65, 1], bf16)
    nc.gpsimd.memset(ones_t[:], 1.0)
    psO = psum_o.tile([1, NS], f32)
    for _ in range(WARM_N):
        nc.tensor.matmul(psO[:, :], warm[:, :1], warm[:, :], start=True, stop=True)

    # --- bit-unpack on DVE: 24 ops, (b, s) pipelined --------------------------
    # m16[b] cols [s*1024:(s+1)*1024] = (words_b << t) & 0x4040  (int16 lanes)
    w16 = wsb[:].bitcast(i16)                       # [128, 3072]
    m16 = []
    for b in range(WBIT):
        mb = sb.tile([128, 8 * 1024], i16, tag=f"m{b}", name=f"m{b}")
        m16.append(mb)
    for b in range(WBIT):
        src = w16[:, 1024 * b : 1024 * (b + 1)]
        for s in range(8):
            dst = m16[b][:, 1024 * s : 1024 * (s + 1)]
            if s < 7:
                nc.vector.tensor_scalar(
                    dst, src, 6 - s, 0x4040,
                    mybir.AluOpType.logical_shift_left,
                    mybir.AluOpType.bitwise_and,
                )
            else:
                nc.vector.tensor_scalar(
                    dst, src, 1, 0x4040,
                    mybir.AluOpType.logical_shift_right,
                    mybir.AluOpType.bitwise_and,
                )

    # --- 48 DoubleRow matmuls: psum[64b+32h+g, n'] = 2*sum_k Qh_k*bit ---------
    # lhsT pair dim = k-tiles (j, j+8), j = 16*c2 + 8*cp + s; cols = h*32+g.
    xv = xsb[:].rearrange("p (c2 cp s hg) -> p c2 cp s hg", c2=2, cp=2, s=8, hg=64)
    ps = [
        psum_a.tile([64, NS], f32, tag="ps0", name="ps0"),
        psum_b.tile([64, NS], f32, tag="ps1", name="ps1"),
        psum_c.tile([64, NS], f32, tag="ps2", name="ps2"),
    ]
    for b in range(WBIT):
        dst = ps[b][:, :]
        for s in range(8):
            mv = m16[b][:, 1024 * s : 1024 * (s + 1)].bitcast(f8)  # [128, 2048]
            mv = mv.rearrange("p (n c) -> p c n", c=4)             # [128, 4, 512]
            for c2 in range(2):
                nc.tensor.matmul(
                    dst,
                    xv[:, c2, :, s, :],
                    mv[:, 2 * c2 : 2 * c2 + 2, :],
                    start=(s == 0 and c2 == 0),
                    stop=(s == 7 and c2 == 1),
                    perf_mode=DR,
                )

    # --- tail: scale-mult (DVE) then ones^T @ prod (PE), bias2 as 65th row ----
    pr0 = sb.tile([64, NS], bf16)
    pr1 = sb.tile([64, NS], bf16)
    nc.vector.tensor_tensor(
        pr0[:, :], ps[0][:, :], csb[:, 0:NS], mybir.AluOpType.mult
    )
    nc.vector.tensor_tensor(
        pr1[:, :], ps[1][:, :], csb[:, NS : 2 * NS], mybir.AluOpType.mult
    )
    nc.vector.tensor_tensor(
        pr2[0:64, :], ps[2][:, :], csb[:, 2 * NS : 3 * NS], mybir.AluOpType.mult
    )
    nc.tensor.matmul(psO[:, :], ones_t[0:64, :], pr0[:, :], start=True, stop=False)
    nc.tensor.matmul(psO[:, :], ones_t[0:64, :], pr1[:, :], start=False, stop=False)
    nc.tensor.matmul(psO[:, :], ones_t[0:65, :], pr2[:, :], start=False, stop=True)
    out_sb = sb.tile([1, NS], f32)
    nc.scalar.copy(out_sb[:], psO[:, :])
    nc.sync.dma_start(out[0:1, :], out_sb[:])


def _get_nc():
    if "nc" not in _CACHE:
        nc = bacc.Bacc(
            "TRN2",
            target_bir_lowering=False,
            debug=False,
            enable_asserts=False,
            num_devices=1,
        )
        with tile.TileContext(nc) as tc:
            _build_kernel_body(tc)
        nc.compile()
        _CACHE["nc"] = nc
    return _CACHE["nc"]


def _prep_inputs(x, binaryWeight, alpha, bias):
    """Host-side shard + layout/encoding prep."""
    x = np.asarray(x, dtype=np.float32).reshape(K)
    binaryWeight = np.asarray(binaryWeight, dtype=np.int32)
    alpha = np.asarray(alpha, dtype=np.int32)
    bias = np.asarray(bias, dtype=np.float32).reshape(N)

    # fp8 main + residual encodings of x
    q0 = x.astype(F8).astype(np.float32)
    q1 = (x - q0).astype(F8).astype(np.float32)

    # Stationary bank: xall[w, j*64 + h*32 + g] = Qh(x[32w+j]) iff g == w//4
    xall = np.zeros((WORDS, G, 2, G), dtype=np.float32)  # [w, j, h, g]
    w = np.arange(WORDS)
    for j in range(G):
        xall[w, j, 0, w // 4] = q0[32 * w + j]
        xall[w, j, 1, w // 4] = q1[32 * w + j]
    xall = xall.reshape(WORDS, 2048).astype(F8)

    # Group sums of the quantized x (for the {-1,+1} offset fix-up)
    sg = (q0 + q1).reshape(G, GROUP).sum(axis=1)  # [G]

    # scale[n, b, g] = sum_a 2^alpha (exact in bf16)
    scale = np.exp2(alpha.astype(np.float32)).sum(axis=-1)  # [N, WBIT, G]

    in_maps = []
    for cc in range(NCORES):
        nsl = slice(cc * NS, (cc + 1) * NS)
        bw_sh = np.ascontiguousarray(binaryWeight[:, :, nsl]).reshape(
            WORDS, WBIT * NS
        )
        sc = scale[nsl]  # [NS, WBIT, G]
        consts = np.zeros((64, 3 * NS), dtype=np.float32)
        # partition 32h+g, cols b*NS:(b+1)*NS  (same scale for h=0 and h=1)
        for b in range(WBIT):
            for h in range(2):
                consts[32 * h : 32 * h + 32, b * NS : (b + 1) * NS] = sc[:, b, :].T
        b2 = bias[nsl] - np.einsum("nbg,g->n", sc, sg)
        in_maps.append(
            {
                "bw": bw_sh,
                "xall": xall,
                "consts": consts.astype(BF16),
                "bias2": b2.reshape(1, NS).astype(BF16),
            }
        )
    return in_maps


def _run(inputs, trace=False, **kw):
    nc = _get_nc()
    in_maps = _prep_inputs(**inputs)
    res = run_bass_kernel_spmd(
        nc, in_maps, core_ids=list(range(NCORES)), trace=trace, **kw
    )
    outs = [res.results[cc]["out"].reshape(NS) for cc in range(NCORES)]
    full = np.concatenate(outs).reshape(1, N).astype(np.float32)
    return full, res


def kernel(**inputs):
    out, _ = _run(inputs, trace=False)
    return out


# revision 12
# speedup vs baseline: 1.0407x; 1.0407x over previous
"""Trainium2 Bass kernel for nn_LutLinear (BCQ/LUT-quantized linear layer).

Math (K=4096, N=4096, WBIT=3, GROUP=128, APOT=3):
  bits[k, b, n]  = bit (k%32) of binaryWeight[k//32, b, n]
  B              = 2*bits - 1                        (in {-1, +1})
  scale[n, b, g] = sum_a 2^alpha[n, b, g, a]
  out[n] = sum_{g,b} scale[n,b,g] * (sum_{k in group g} x[k] * B[k,b,n]) + bias[n]

Strategy (tensor-parallel over N, 8 cores, N'=512 each), raw bass (no Tile
framework -- manual semaphores, so the epilogue semaphore-clear churn that
dominated the Tile version's teardown disappears):
  * DVE bit-unpack: (words << t) & 0x4040 on int16 lanes yields fp8e4 planes
    (byte 0x40 = 2.0), 8 whole-tile ops.
  * PE: 96 matmuls lhsT = block-diagonal x bank (bf16) [128, 32], rhs = fp8
    bit-plane view [128, 512] (stride 4).  The 3 b-matmuls per (s, c) target
    psum col-blocks 0/32/64 and column-tile 3-way on the array.
  * Tail: prod[q, n'] = psum96 * scale (bf16, one DVE op), ones^T @ prod on
    PE (97th row = bias2 = bias - sum_q scale*S_g), DVE copy psum->SBUF, DMA.
"""

import os
import sys

for _p in ("/opt/trn_rl_repo", "/opt/pypackages"):
    if os.path.isdir(_p) and _p not in sys.path:
        sys.path.insert(0, _p)

from contextlib import ExitStack

import ml_dtypes
import numpy as np

import concourse.bass as bass
from concourse import bacc, mybir
from concourse.bass_utils import run_bass_kernel_spmd

K = 4096
N = 4096
GROUP = 128
WBIT = 3
G = K // GROUP          # 32 groups
NCORES = 8
NS = N // NCORES        # 512 output features per core
WORDS = K // 32         # 128 packed words per (b, n)
Q = WBIT * G            # 96 psum rows
BF16 = ml_dtypes.bfloat16

_CACHE = {}


def _build(nc):
    f32 = mybir.dt.float32
    i32 = mybir.dt.int32
    i16 = mybir.dt.int16
    bf16 = mybir.dt.bfloat16
    f8 = mybir.dt.float8e4
    LSL = mybir.AluOpType.logical_shift_left
    LSR = mybir.AluOpType.logical_shift_right
    AND = mybir.AluOpType.bitwise_and

    bw = nc.dram_tensor("bw", [WORDS, WBIT * NS], i32, kind="ExternalInput")
    xall = nc.dram_tensor("xall", [WORDS, G * G], bf16, kind="ExternalInput")
    consts = nc.dram_tensor("consts", [Q, NS], bf16, kind="ExternalInput")
    bias2 = nc.dram_tensor("bias2", [1, NS], bf16, kind="ExternalInput")
    out = nc.dram_tensor("out", [1, NS], f32, kind="ExternalOutput")

    ctx = ExitStack()
    wsb = ctx.enter_context(nc.sbuf_tensor("wsb", [WORDS, WBIT * NS], i32))
    m16 = ctx.enter_context(nc.sbuf_tensor("m16", [128, 8 * 3072], i16))
    xsb = ctx.enter_context(nc.sbuf_tensor("xsb", [WORDS, G * G], bf16))
    csb = ctx.enter_context(nc.sbuf_tensor("csb", [Q, NS], bf16))
    pr = ctx.enter_context(nc.sbuf_tensor("pr", [Q + 1, NS], bf16))
    warm = ctx.enter_context(nc.sbuf_tensor("warm", [128, 512], bf16))
    ones = ctx.enter_context(nc.sbuf_tensor("ones", [Q + 1, 1], bf16))
    outsb = ctx.enter_context(nc.sbuf_tensor("outsb", [1, NS], f32))
    ps96 = ctx.enter_context(nc.psum_tensor("ps96", [Q, NS], f32))
    psO = ctx.enter_context(nc.psum_tensor("psO", [1, NS], f32))

    s_bw = ctx.enter_context(nc.semaphore("s_bw"))
    s_b2 = ctx.enter_context(nc.semaphore("s_b2"))
    s_x = ctx.enter_context(nc.semaphore("s_x"))
    s_cs = ctx.enter_context(nc.semaphore("s_cs"))
    s_pool = ctx.enter_context(nc.semaphore("s_pool"))
    s_up = ctx.enter_context(nc.semaphore("s_up"))
    s_mm = ctx.enter_context(nc.semaphore("s_mm"))
    s_pr = ctx.enter_context(nc.semaphore("s_pr"))
    s_red = ctx.enter_context(nc.semaphore("s_red"))
    s_out = ctx.enter_context(nc.semaphore("s_out"))
    s_done = ctx.enter_context(nc.semaphore("s_done"))

    # Re-run safety: clear kernel semaphores before any engine proceeds.
    sem_nums = sorted(
        s.num
        for s in (s_bw, s_b2, s_x, s_cs, s_pool, s_up, s_mm, s_pr, s_red, s_out, s_done)
    )
    for rng in _compact_ranges(sem_nums):
        nc.gpsimd.dma_reset(rng)
        nc.gpsimd.sem_clear(rng)
    nc._nrt_pseudo_barrier()

    w16 = wsb[:].bitcast(i16)                       # [128, 3072]
    xv = xsb[:].rearrange("p (j g) -> p j g", j=G)  # [128, 32, 32]

    with nc.Block(no_gpsimd_drain=True) as block:

        @block.sync
        def _(sync):
            sync.dma_start(wsb[:], bw[:, :]).then_inc(s_bw, 16)
            sync.dma_start(pr[Q : Q + 1, :], bias2[0:1, :]).then_inc(s_b2, 16)
            sync.wait_ge(s_out, 1)
            sync.dma_start(out[0:1, :], outsb[:]).then_inc(s_done, 16)
            sync.wait_ge(s_done, 16)

        @block.scalar
        def _(scalar):
            scalar.dma_start(xsb[:], xall[:, :]).then_inc(s_x, 16)
            scalar.dma_start(csb[:], consts[:, :]).then_inc(s_cs, 16)

        @block.gpsimd
        def _(gpsimd):
            gpsimd.memset(warm[:], 0.0).then_inc(s_pool, 1)
            gpsimd.memset(ones[:], 1.0).then_inc(s_pool, 1)

        @block.vector
        def _(vector):
            vector.wait_ge(s_bw, 16)
            for s in range(8):
                dst = m16[:, 3072 * s : 3072 * (s + 1)]
                if s < 7:
                    vector.tensor_scalar(dst, w16, 6 - s, 0x4040, LSL, AND).then_inc(
                        s_up, 1
                    )
                else:
                    vector.tensor_scalar(dst, w16, 1, 0x4040, LSR, AND).then_inc(
                        s_up, 1
                    )
            vector.wait_ge(s_mm, WBIT)
            vector.wait_ge(s_cs, 16)
            vector.tensor_tensor(
                pr[0:Q, :], ps96[:], csb[:], mybir.AluOpType.mult
            ).then_inc(s_pr, 1)
            vector.wait_ge(s_red, 1)
            vector.tensor_scalar(
                outsb[:], psO[:], 0.0, None, mybir.AluOpType.add
            ).then_inc(s_out, 1)

        @block.tensor
        def _(tensor):
            tensor.wait_ge(s_pool, 1)
            wf32 = warm[:].bitcast(f32)             # [128, 256]
            tensor.matmul(
                psO[0:1, 0:256], wf32[:, 0:1], wf32[:, :], start=True, stop=True
            )
            tensor.matmul(
                psO[0:1, 0:256], wf32[:, 0:1], wf32[:, :], start=True, stop=True
            )
            tensor.matmul(
                psO[0:1, 0:512], warm[:, 0:1], warm[:, :], start=True, stop=True
            )
            tensor.wait_ge(s_x, 16)
            for s in range(8):
                mv = m16[:, 3072 * s : 3072 * (s + 1)].bitcast(f8)
                mv = mv.rearrange("p (b n c) -> p b c n", b=WBIT, n=NS, c=4)
                tensor.wait_ge(s_up, s + 1)
                for c in range(4):
                    j = 8 * c + s
                    for b in range(WBIT):
                        mm = tensor.matmul(
                            ps96[32 * b : 32 * b + 32, :],
                            xv[:, j, :],
                            mv[:, b, c, :],
                            start=(s == 0 and c == 0),
                            stop=(s == 7 and c == 3),
                            skip_group_check=True,
                        )
                        if s == 7 and c == 3:
                            mm.then_inc(s_mm, 1)
            tensor.wait_ge(s_pr, 1)
            tensor.wait_ge(s_b2, 16)
            tensor.wait_ge(s_pool, 2)
            tensor.matmul(
                psO[0:1, :], ones[:, :], pr[:, :], start=True, stop=True
            ).then_inc(s_red, 1)

    ctx.close()


def _compact_ranges(nums):
    out = []
    start = prev = nums[0]
    for n in nums[1:]:
        if n == prev + 1:
            prev = n
            continue
        out.append(range(start, prev + 1))
        start = prev = n
    out.append(range(start, prev + 1))
    return out


def _get_nc():
    if "nc" not in _CACHE:
        nc = bacc.Bacc(
            "TRN2",
            target_bir_lowering=False,
            debug=False,
            enable_asserts=False,
            num_devices=1,
        )
        _build(nc)
        nc.compile()
        _CACHE["nc"] = nc
    return _CACHE["nc"]


def _prep_inputs(x, binaryWeight, alpha, bias):
    """Host-side shard + layout/encoding prep."""
    x = np.asarray(x, dtype=np.float32).reshape(K)
    binaryWeight = np.asarray(binaryWeight, dtype=np.int32)
    alpha = np.asarray(alpha, dtype=np.int32)
    bias = np.asarray(bias, dtype=np.float32).reshape(N)

    # Block-diagonal lhsT bank: xall[w, j*32 + g] = x[32w + j] iff g == w//4
    xall = np.zeros((WORDS, G, G), dtype=np.float32)  # [w, j, g]
    w = np.arange(WORDS)
    for j in range(G):
        xall[w, j, w // 4] = x[32 * w + j]
    xallb = xall.reshape(WORDS, G * G).astype(BF16)

    xb = xallb.astype(np.float32)
    sg = xb.reshape(WORDS, G, G).sum(axis=(0, 1))  # effective group sums [G]

    # scale[n, b, g] = sum_a 2^alpha (exact in bf16)
    scale = np.exp2(alpha.astype(np.float32)).sum(axis=-1)  # [N, WBIT, G]

    in_maps = []
    for cc in range(NCORES):
        nsl = slice(cc * NS, (cc + 1) * NS)
        bw_sh = np.ascontiguousarray(binaryWeight[:, :, nsl]).reshape(
            WORDS, WBIT * NS
        )
        sc = scale[nsl]  # [NS, WBIT, G]
        consts = np.zeros((Q, NS), dtype=np.float32)
        for b in range(WBIT):
            consts[32 * b : 32 * b + 32, :] = sc[:, b, :].T
        b2 = bias[nsl] - np.einsum("nbg,g->n", sc, sg)
        in_maps.append(
            {
                "bw": bw_sh,
                "xall": xallb,
                "consts": consts.astype(BF16),
                "bias2": b2.reshape(1, NS).astype(BF16),
            }
        )
    return in_maps


def _run(inputs, trace=False, **kw):
    nc = _get_nc()
    in_maps = _prep_inputs(**inputs)
    res = run_bass_kernel_spmd(
        nc, in_maps, core_ids=list(range(NCORES)), trace=trace, **kw
    )
    outs = [res.results[cc]["out"].reshape(NS) for cc in range(NCORES)]
    full = np.concatenate(outs).reshape(1, N).astype(np.float32)
    return full, res


def kernel(**inputs):
    out, _ = _run(inputs, trace=False)
    return out


# revision 13
# speedup vs baseline: 1.1202x; 1.0764x over previous
"""Trainium2 Bass kernel for nn_LutLinear (BCQ/LUT-quantized linear layer).

Math (K=4096, N=4096, WBIT=3, GROUP=128, APOT=3):
  bits[k, b, n]  = bit (k%32) of binaryWeight[k//32, b, n]
  B              = 2*bits - 1                        (in {-1, +1})
  scale[n, b, g] = sum_a 2^alpha[n, b, g, a]
  out[n] = sum_{g,b} scale[n,b,g] * (sum_{k in group g} x[k] * B[k,b,n]) + bias[n]

Strategy (tensor-parallel over N, 8 cores, N'=512 each), raw bass (no Tile
framework -- manual semaphores, so the epilogue semaphore-clear churn that
dominated the Tile version's teardown disappears):
  * DVE bit-unpack: (words << t) & 0x4040 on int16 lanes yields fp8e4 planes
    (byte 0x40 = 2.0), 8 whole-tile ops.
  * PE: 96 matmuls lhsT = block-diagonal x bank (bf16) [128, 32], rhs = fp8
    bit-plane view [128, 512] (stride 4).  The 3 b-matmuls per (s, c) target
    psum col-blocks 0/32/64 and column-tile 3-way on the array.
  * Tail: prod[q, n'] = psum96 * scale (bf16, one DVE op), ones^T @ prod on
    PE (97th row = bias2 = bias - sum_q scale*S_g), DVE copy psum->SBUF, DMA.
"""

import os
import sys

for _p in ("/opt/trn_rl_repo", "/opt/pypackages"):
    if os.path.isdir(_p) and _p not in sys.path:
        sys.path.insert(0, _p)

from contextlib import ExitStack

import ml_dtypes
import numpy as np

import concourse.bass as bass
from concourse import bacc, mybir
from concourse.bass_utils import run_bass_kernel_spmd

K = 4096
N = 4096
GROUP = 128
WBIT = 3
G = K // GROUP          # 32 groups
NCORES = 8
NS = N // NCORES        # 512 output features per core
WORDS = K // 32         # 128 packed words per (b, n)
Q = WBIT * G            # 96 psum rows
BF16 = ml_dtypes.bfloat16

_CACHE = {}


def _build(nc):
    f32 = mybir.dt.float32
    i32 = mybir.dt.int32
    i16 = mybir.dt.int16
    bf16 = mybir.dt.bfloat16
    f8 = mybir.dt.float8e4
    LSL = mybir.AluOpType.logical_shift_left
    LSR = mybir.AluOpType.logical_shift_right
    AND = mybir.AluOpType.bitwise_and

    bw = nc.dram_tensor("bw", [WORDS, WBIT * NS], i32, kind="ExternalInput")
    xall = nc.dram_tensor("xall", [WORDS, G * G], bf16, kind="ExternalInput")
    consts = nc.dram_tensor("consts", [Q, NS], bf16, kind="ExternalInput")
    bias2 = nc.dram_tensor("bias2", [1, NS], bf16, kind="ExternalInput")
    out = nc.dram_tensor("out", [1, NS], f32, kind="ExternalOutput")

    ctx = ExitStack()
    wsb = ctx.enter_context(nc.sbuf_tensor("wsb", [WORDS, WBIT * NS], i32))
    m16 = ctx.enter_context(nc.sbuf_tensor("m16", [128, 8 * 3072], i16))
    xsb = ctx.enter_context(nc.sbuf_tensor("xsb", [WORDS, G * G], bf16))
    csb = ctx.enter_context(nc.sbuf_tensor("csb", [Q, NS], bf16))
    pr = ctx.enter_context(nc.sbuf_tensor("pr", [Q + 1, NS], bf16))
    warm = ctx.enter_context(nc.sbuf_tensor("warm", [128, 1024], bf16))
    ones = ctx.enter_context(nc.sbuf_tensor("ones", [Q + 1, 1], bf16))
    outsb = ctx.enter_context(nc.sbuf_tensor("outsb", [1, NS], f32))
    ps96 = ctx.enter_context(nc.psum_tensor("ps96", [Q, NS], f32))
    psO = ctx.enter_context(nc.psum_tensor("psO", [1, NS], f32))

    s_bw = ctx.enter_context(nc.semaphore("s_bw"))
    s_b2 = ctx.enter_context(nc.semaphore("s_b2"))
    s_x = ctx.enter_context(nc.semaphore("s_x"))
    s_cs = ctx.enter_context(nc.semaphore("s_cs"))
    s_pool = ctx.enter_context(nc.semaphore("s_pool"))
    s_up = ctx.enter_context(nc.semaphore("s_up"))
    s_mm = ctx.enter_context(nc.semaphore("s_mm"))
    s_pr = ctx.enter_context(nc.semaphore("s_pr"))
    s_red = ctx.enter_context(nc.semaphore("s_red"))
    s_out = ctx.enter_context(nc.semaphore("s_out"))
    s_done = ctx.enter_context(nc.semaphore("s_done"))

    # Re-run safety: clear kernel semaphores before any engine proceeds.
    sem_nums = sorted(
        s.num
        for s in (s_bw, s_b2, s_x, s_cs, s_pool, s_up, s_mm, s_pr, s_red, s_out, s_done)
    )
    for rng in _compact_ranges(sem_nums):
        nc.gpsimd.dma_reset(rng)
        nc.gpsimd.sem_clear(rng)
    nc._nrt_pseudo_barrier()

    w16 = wsb[:].bitcast(i16)                       # [128, 3072]
    xv = xsb[:].rearrange("p (j g) -> p j g", j=G)  # [128, 32, 32]

    with nc.Block(no_gpsimd_drain=True) as block:

        @block.sync
        def _(sync):
            sync.dma_start(wsb[:], bw[:, :]).then_inc(s_bw, 16)
            sync.dma_start(pr[Q : Q + 1, :], bias2[0:1, :]).then_inc(s_b2, 16)
            sync.wait_ge(s_out, 1)
            sync.dma_start(out[0:1, :], outsb[:]).then_inc(s_done, 16)
            sync.wait_ge(s_done, 16)

        @block.scalar
        def _(scalar):
            scalar.dma_start(xsb[:], xall[:, :]).then_inc(s_x, 16)
            scalar.dma_start(csb[:], consts[:, :]).then_inc(s_cs, 16)

        @block.gpsimd
        def _(gpsimd):
            gpsimd.memset(warm[:], 0.0).then_inc(s_pool, 1)
            gpsimd.memset(ones[:], 1.0).then_inc(s_pool, 1)

        @block.vector
        def _(vector):
            vector.wait_ge(s_bw, 16)
            for s in range(8):
                dst = m16[:, 3072 * s : 3072 * (s + 1)]
                if s < 7:
                    vector.tensor_scalar(dst, w16, 6 - s, 0x4040, LSL, AND).then_inc(
                        s_up, 1
                    )
                else:
                    vector.tensor_scalar(dst, w16, 1, 0x4040, LSR, AND).then_inc(
                        s_up, 1
                    )
            vector.wait_ge(s_mm, WBIT)
            vector.wait_ge(s_cs, 16)
            vector.tensor_tensor(
                pr[0:Q, :], ps96[:], csb[:], mybir.AluOpType.mult
            ).then_inc(s_pr, 1)
            vector.wait_ge(s_red, 1)
            vector.tensor_scalar(
                outsb[:], psO[:], 0.0, None, mybir.AluOpType.add
            ).then_inc(s_out, 1)

        @block.tensor
        def _(tensor):
            tensor.wait_ge(s_pool, 1)
            wf32 = warm[:].bitcast(f32)             # [128, 512]
            tensor.matmul(
                psO[0:1, 0:512], wf32[:, 0:1], wf32[:, :], start=True, stop=True
            )
            tensor.matmul(
                psO[0:1, 0:512], wf32[:, 0:1], wf32[:, :], start=True, stop=True
            )
            tensor.matmul(
                psO[0:1, 0:512], warm[:, 0:1], warm[:, 0:512], start=True, stop=True
            )
            tensor.wait_ge(s_x, 16)
            for s in range(8):
                mv = m16[:, 3072 * s : 3072 * (s + 1)].bitcast(f8)
                mv = mv.rearrange("p (b n c) -> p b c n", b=WBIT, n=NS, c=4)
                tensor.wait_ge(s_up, s + 1)
                for c in range(4):
                    j = 8 * c + s
                    for b in range(WBIT):
                        mm = tensor.matmul(
                            ps96[32 * b : 32 * b + 32, :],
                            xv[:, j, :],
                            mv[:, b, c, :],
                            start=(s == 0 and c == 0),
                            stop=(s == 7 and c == 3),
                            skip_group_check=True,
                        )
                        if s == 7 and c == 3:
                            mm.then_inc(s_mm, 1)
            tensor.wait_ge(s_pr, 1)
            tensor.wait_ge(s_b2, 16)
            tensor.wait_ge(s_pool, 2)
            tensor.matmul(
                psO[0:1, :], ones[:, :], pr[:, :], start=True, stop=True
            ).then_inc(s_red, 1)

    ctx.close()


def _compact_ranges(nums):
    out = []
    start = prev = nums[0]
    for n in nums[1:]:
        if n == prev + 1:
            prev = n
            continue
        out.append(range(start, prev + 1))
        start = prev = n
    out.append(range(start, prev + 1))
    return out


def _get_nc():
    if "nc" not in _CACHE:
        nc = bacc.Bacc(
            "TRN2",
            target_bir_lowering=False,
            debug=False,
            enable_asserts=False,
            num_devices=1,
        )
        _build(nc)
        nc.compile()
        _CACHE["nc"] = nc
    return _CACHE["nc"]


def _prep_inputs(x, binaryWeight, alpha, bias):
    """Host-side shard + layout/encoding prep."""
    x = np.asarray(x, dtype=np.float32).reshape(K)
    binaryWeight = np.asarray(binaryWeight, dtype=np.int32)
    alpha = np.asarray(alpha, dtype=np.int32)
    bias = np.asarray(bias, dtype=np.float32).reshape(N)

    # Block-diagonal lhsT bank: xall[w, j*32 + g] = x[32w + j] iff g == w//4
    xall = np.zeros((WORDS, G, G), dtype=np.float32)  # [w, j, g]
    w = np.arange(WORDS)
    for j in range(G):
        xall[w, j, w // 4] = x[32 * w + j]
    xallb = xall.reshape(WORDS, G * G).astype(BF16)

    xb = xallb.astype(np.float32)
    sg = xb.reshape(WORDS, G, G).sum(axis=(0, 1))  # effective group sums [G]

    # scale[n, b, g] = sum_a 2^alpha (exact in bf16)
    scale = np.exp2(alpha.astype(np.float32)).sum(axis=-1)  # [N, WBIT, G]

    in_maps = []
    for cc in range(NCORES):
        nsl = slice(cc * NS, (cc + 1) * NS)
        bw_sh = np.ascontiguousarray(binaryWeight[:, :, nsl]).reshape(
            WORDS, WBIT * NS
        )
        sc = scale[nsl]  # [NS, WBIT, G]
        consts = np.zeros((Q, NS), dtype=np.float32)
        for b in range(WBIT):
            consts[32 * b : 32 * b + 32, :] = sc[:, b, :].T
        b2 = bias[nsl] - np.einsum("nbg,g->n", sc, sg)
        in_maps.append(
            {
                "bw": bw_sh,
                "xall": xallb,
                "consts": consts.astype(BF16),
                "bias2": b2.reshape(1, NS).astype(BF16),
            }
        )
    return in_maps


def _run(inputs, trace=False, **kw):
    nc = _get_nc()
    in_maps = _prep_inputs(**inputs)
    res = run_bass_kernel_spmd(
        nc, in_maps, core_ids=list(range(NCORES)), trace=trace, **kw
    )
    outs = [res.results[cc]["out"].reshape(NS) for cc in range(NCORES)]
    full = np.concatenate(outs).reshape(1, N).astype(np.float32)
    return full, res


def kernel(**inputs):
    out, _ = _run(inputs, trace=False)
    return out
